# revision 19
# baseline (speedup 1.0000x reference)
"""BiLSTM-CRF Viterbi decode on Trainium2 (Bass/Tile).

Single-core compute graph, run SPMD-redundantly on 8 NeuronCores (each core
computes the full result on identical inputs; core 0's output is returned).

Stages:
  A. embedding gather (indirect DMA) -> X ; PE-transpose -> X.T ; bf16 hi/lo split;
     zx = X @ Wih'.T + b'  (3-product hi/lo bf16 matmuls = ~fp32 precision)
     -> zx DRAM, gate-column-permuted, sigma-only prescaling folded in.
  B. two phase-shifted sequential LSTM recurrences (fwd rows {0,32}, bwd {64,96}):
     per step: zx round + 6 weight rounds (p3) into PSUM -> sigma(z) ->
     fused cell update (scalar_tensor_tensor) -> h/2 -> PE transpose ->
     bf16 hi/lo state for next step's stationary + hs log (partition-major).
  C. feats = hs @ (2*W_out).T (p3) -> (48, S) partition-major.
  D. Viterbi forward, exact reference op-order: tmp=trans+prev ; PE transpose ;
     +obs ; max8 + max_index backpointers.  (bit-exact vs jax fp32 on CPU)
  E. backtrack as one-hot matvec chain on PE (exact integers); path = iota @ V.
"""

import numpy as np
import ml_dtypes

V, E, H, T = 100000, 256, 512, 48
HD = H // 2
G4 = 4 * HD

bf16 = ml_dtypes.bfloat16

# gate-column permutation: [i.lo f.lo g.lo o.lo | i.hi f.hi g.hi o.hi] (128 each)
PERM = np.r_[0:128, 256:384, 512:640, 768:896, 128:256, 384:512, 640:768, 896:1024]


def _split_hilo(x):
    hi = x.astype(bf16)
    lo = (x - hi.astype(np.float32)).astype(bf16)
    return hi, lo


def _pack_k(w_t):
    """(256, C) -> (128, 2, C) with [p, c, :] = w_t[c*128+p, :]"""
    return np.ascontiguousarray(w_t.reshape(2, 128, -1).transpose(1, 0, 2))


def prep_shared(inputs):
    """Weights/consts shared by all cores (window-independent)."""
    f32 = np.float32
    out = {"embed": np.ascontiguousarray(np.asarray(inputs["embed_table"], f32))}

    wt_hi, wt_lo, wit_hi, wit_lo, brow = [], [], [], [], []
    for d in ("f", "b"):
        Whh = np.asarray(inputs[f"Whh_{d}"], f32)
        Wih = np.asarray(inputs[f"Wih_{d}"], f32)
        b = np.asarray(inputs[f"b_{d}"], f32).copy()
        gscale = np.ones((G4, 1), f32)
        gscale[2 * HD:3 * HD] = 2.0          # sigma-form tanh: g rows x2
        Wrec = (2.0 * Whh) * gscale          # x2 again: state is h/2
        Wzx = Wih * gscale
        b2 = b * gscale[:, 0]
        hi, lo = _split_hilo(_pack_k(np.ascontiguousarray(Wrec.T[:, PERM])))
        wt_hi.append(hi); wt_lo.append(lo)
        hi, lo = _split_hilo(_pack_k(np.ascontiguousarray(Wzx.T[:, PERM])))
        wit_hi.append(hi); wit_lo.append(lo)
        brow.append(b2[PERM])
    out["WT_hi"] = np.ascontiguousarray(np.concatenate(wt_hi, -1))    # (128,2,2048)
    out["WT_lo"] = np.ascontiguousarray(np.concatenate(wt_lo, -1))
    out["WIT_hi"] = np.ascontiguousarray(np.concatenate(wit_hi, -1))
    out["WIT_lo"] = np.ascontiguousarray(np.concatenate(wit_lo, -1))
    out["bias_row"] = np.concatenate(brow)[None, :].astype(f32)       # (1, 2048)

    W_out2 = 2.0 * np.asarray(inputs["W_out"], f32)
    woh, wol = _split_hilo(
        np.ascontiguousarray(W_out2.T).reshape(4, 128, 48).transpose(1, 0, 2))
    out["WOT_hi"] = np.ascontiguousarray(woh)                          # (128,4,48)
    out["WOT_lo"] = np.ascontiguousarray(wol)

    out["trans"] = np.ascontiguousarray(np.asarray(inputs["transitions"], f32))

    # consts (128, 512):
    # 0:2 I2 at rows {0,32},{64,96}; 2:50 I48; 50:178 ones row0; 178:226 iota row0;
    # 226 iota col; 240:288 iotaM[k,i]=i; 320:448 I128
    consts = np.zeros((128, 512), f32)
    consts[0, 0] = 1.0
    consts[32, 1] = 1.0
    consts[0:48, 2:50] = np.eye(48, dtype=f32)
    consts[0, 50:178] = 1.0
    consts[0, 178:226] = np.arange(48, dtype=f32)
    consts[0:48, 226] = np.arange(48, dtype=f32)
    consts[0:48, 240:288] = np.arange(48, dtype=f32)[None, :]
    for r0 in (0, 32, 64, 96):
        consts[r0, 289] = 1.0
    consts[:, 320:448] = np.eye(128, dtype=f32)
    out["consts"] = consts
    return out


def prep_core(inputs, ws, S, fwd_real, bwd_real):
    """Per-core window inputs: sent slice + effective h0/c0 (zero for warmup
    chains, real initial state only at the true sequence edges)."""
    f32 = np.float32
    sent = np.asarray(inputs["sent"]).astype(np.int32)[ws:ws + S]
    h0 = np.asarray(inputs["h0"], f32).copy()
    c0 = np.asarray(inputs["c0"], f32).copy()
    if not fwd_real:
        h0[0] = 0.0; c0[0] = 0.0
    if not bwd_real:
        h0[1] = 0.0; c0[1] = 0.0
    h0 = h0 / 2.0
    out = {"sent": sent.reshape(S // 128, 128).T.copy()}  # [p, w] = sent[w*128+p]
    hblk = []
    for d in range(2):
        hi, lo = _split_hilo(np.ascontiguousarray(h0[d].reshape(2, 128).T))
        hblk.append(np.concatenate([hi, lo], 1))
    out["h0_init"] = np.ascontiguousarray(np.concatenate(hblk, 1))     # (128,8)
    out["c0_init"] = np.ascontiguousarray(
        np.concatenate([c0[0].reshape(2, 128), c0[1].reshape(2, 128)], 0))  # (4,128)
    return out


def emit(nc, tc, io, S, WIN, stages="ABCDE"):
    import concourse.bass as bass
    import concourse.mybir as mybir

    fp32 = mybir.dt.float32
    bf = mybir.dt.bfloat16
    u16 = mybir.dt.uint16
    AF = mybir.ActivationFunctionType
    OP = mybir.AluOpType

    assert S % 128 == 0 and WIN % 4 == 0 and S % WIN == 0
    NW = S // WIN
    SW = S // 128

    persist = tc.alloc_tile_pool(name="persist", bufs=1)
    pt = lambda shape, dt, name: persist.tile(shape, dt, name=name)

    consts = pt([128, 512], fp32, "consts")
    WT_hi = pt([128, 2 * 2048], bf, "WT_hi_s")
    WT_lo = pt([128, 2 * 2048], bf, "WT_lo_s")
    hs = pt([128, 8 * S], bf, "hs")
    hstate = pt([128, 16], bf, "hstate")
    mx = pt([48, 8], fp32, "mx")
    bps = pt([48, S + 8], u16, "bps")     # col t-1 <- idx0 of step t (overlapped writes)
    tmp = pt([48, 48], fp32, "tmp")
    tmp2 = pt([48, 48], fp32, "tmp2")
    Mscr = pt([48, 4 * 48], fp32, "Mscr")
    lastrow = pt([1, 48], fp32, "lastrow")
    lmax = pt([1, 8], fp32, "lmax")
    lidx = pt([1, 8], mybir.dt.uint32, "lidx")
    lidxf = pt([1, 1], fp32, "lidxf")
    vrow = pt([1, 48], fp32, "vrow")
    trans_s = pt([48, 48], fp32, "trans_s")
    WOT_hi_s = pt([128, 4 * 48], bf, "WOT_hi_s")
    WOT_lo_s = pt([128, 4 * 48], bf, "WOT_lo_s")

    # per-direction working tiles; rows used: {0, 32}
    sig = [pt([33, 512], fp32, "sigF"), pt([33, 512], fp32, "sigB")]
    cst = [pt([33, 128], fp32, "cstF"), pt([33, 128], fp32, "cstB")]
    igt = [pt([33, 128], fp32, "igF"), pt([33, 128], fp32, "igB")]
    fct = [pt([33, 128], fp32, "fcF"), pt([33, 128], fp32, "fcB")]
    sct = [pt([33, 128], fp32, "scF"), pt([33, 128], fp32, "scB")]
    hht = [pt([33, 128], fp32, "hhF"), pt([33, 128], fp32, "hhB")]

    I2c = consts[0:33, 0:2]
    I48 = consts[0:48, 2:50]
    ones_row = consts[0:1, 50:178]
    one_1 = consts[0:1, 50:51]
    iota_row = consts[0:1, 178:226]
    iota_col = consts[0:48, 226:227]
    iotaM = consts[0:48, 240:288]
    I128 = consts[:, 320:448]

    # shared scratch pools (single pools for the whole program; tags give WAR sync)
    scr = tc.alloc_tile_pool(name="scratch", bufs=1)
    psum = tc.alloc_tile_pool(name="psum", bufs=1, space="PSUM")

    psz = tc.alloc_tile_pool(name="psz", bufs=1, space="PSUM")
    zpst = [[psz.tile([128, 512], fp32, name=f"zper{d}{p}") for p in range(2)]
            for d in range(2)]

    def ps_big(name):    # 2 banks
        return psum.tile([128, 1024], fp32, name=name, tag="big", bufs=1)

    def ps_small(name):  # 1 bank x2
        return psum.tile([128, 128], fp32, name=name, tag="small", bufs=2)

    nc.sync.dma_start(consts[:], io["consts"])
    nc.sync.dma_start(WT_hi[:], io["WT_hi"].rearrange("p c g -> p (c g)"))
    nc.sync.dma_start(WT_lo[:], io["WT_lo"].rearrange("p c g -> p (c g)"))
    nc.sync.dma_start(hstate[:, 0:8], io["h0_init"])
    for d_ in range(2):
        for t_ in (sig[d_], cst[d_], igt[d_], fct[d_], sct[d_], hht[d_]):
            nc.vector.memset(t_[:], 0.0)
        for p_ in range(2):
            nc.vector.memset(zpst[d_][p_][:], 0.0)
    nc.sync.dma_start(cst[0][0:33:32, :], io["c0_init"][0:2, :])
    nc.sync.dma_start(cst[1][0:33:32, :], io["c0_init"][2:4, :])
    nc.sync.dma_start(trans_s[:], io["trans"])
    nc.sync.dma_start(WOT_hi_s[:], io["WOT_hi"].rearrange("p c t -> p (c t)"))
    nc.sync.dma_start(WOT_lo_s[:], io["WOT_lo"].rearrange("p c t -> p (c t)"))

    WTh = WT_hi[:].rearrange("p (c g) -> p c g", c=2)
    WTl = WT_lo[:].rearrange("p (c g) -> p c g", c=2)

    def wt(part, kc, d, strip):
        t = WTh if part == 0 else WTl
        base = d * 1024 + strip * 512
        return t[:, kc, base:base + 512]

    # ---------------- Stage A ----------------
    sentb = scr.tile([128, SW], mybir.dt.int32, name="sentb", tag="sentb")
    X = scr.tile([128, SW * 256], fp32, name="X", tag="gx")
    XT_hi = scr.tile([128, 2 * S], bf, name="XT_hi", tag="sc8a")
    XT_lo = scr.tile([128, 2 * S], bf, name="XT_lo", tag="sc8b")
    bias_s = scr.tile([1, 2048], fp32, name="bias_s", tag="bias")
    WIT_hi_s = scr.tile([128, 2 * 2048], bf, name="WIT_hi_s", tag="sc8c")
    WIT_lo_s = scr.tile([128, 2 * 2048], bf, name="WIT_lo_s", tag="sc8d")

    nc.sync.dma_start(bias_s[:], io["bias_row"])
    nc.sync.dma_start(sentb[:], io["sent"])
    nc.sync.dma_start(WIT_hi_s[:], io["WIT_hi"].rearrange("p c g -> p (c g)"))
    nc.sync.dma_start(WIT_lo_s[:], io["WIT_lo"].rearrange("p c g -> p (c g)"))

    Xv = X[:].rearrange("p (w e) -> p w e", w=SW)
    for w in range(SW):
        nc.gpsimd.indirect_dma_start(
            out=Xv[:, w, :], out_offset=None,
            in_=io["embed"],
            in_offset=bass.IndirectOffsetOnAxis(ap=sentb[:, w:w + 1], axis=0))

    XTh = XT_hi[:].rearrange("p (c t) -> p c t", c=2)
    XTl = XT_lo[:].rearrange("p (c t) -> p c t", c=2)
    for w in range(SW):
        for ec in range(2):
            tps = ps_small("tpsa")
            nc.tensor.transpose(tps[:], Xv[:, w, ec * 128:(ec + 1) * 128], I128)
            nc.vector.tensor_copy(XTh[:, ec, w * 128:(w + 1) * 128], tps[:])
            nc.vector.tensor_tensor(out=XTl[:, ec, w * 128:(w + 1) * 128],
                                    in0=tps[:],
                                    in1=XTh[:, ec, w * 128:(w + 1) * 128],
                                    op=OP.subtract)

    WITh = WIT_hi_s[:].rearrange("p (c g) -> p c g", c=2)
    WITl = WIT_lo_s[:].rearrange("p (c g) -> p c g", c=2)

    for tt in range(SW):
        for gh in range(2):
            zps = ps_big("zxps")
            zsb = scr.tile([128, 1024], fp32, name="zxsb", tag="sc8e", bufs=2)
            for nb in range(2):
                cb = gh * 1024 + nb * 512
                nc.tensor.matmul(zps[:, nb * 512:(nb + 1) * 512], ones_row,
                                 bias_s[0:1, cb:cb + 512],
                                 start=True, stop=False)
            for ri, (Xp, Wp) in enumerate([(XTh, WITh), (XTl, WITh), (XTh, WITl)]):
                for kc in range(2):
                    for nb in range(2):
                        cb = gh * 1024 + nb * 512
                        nc.tensor.matmul(
                            zps[:, nb * 512:(nb + 1) * 512],
                            Xp[:, kc, tt * 128:(tt + 1) * 128],
                            Wp[:, kc, cb:cb + 512],
                            start=False, stop=(ri == 2 and kc == 1))
            nc.vector.tensor_copy(zsb[:], zps[:])
            nc.sync.dma_start(io["zx"][tt * 128:(tt + 1) * 128, gh * 1024:(gh + 1) * 1024],
                              zsb[:])

    tc.strict_bb_all_engine_barrier()

    # ---------------- Stage B ----------------
    def r2(tile_, cols):
        return tile_[0:33, cols]

    def lstm_mm(d, j, gpar, zwin_t):
        st8 = gpar * 8 + d * 4
        zps = zpst[d][gpar]
        jp = 32 * (j % 4)
        jb = (j // 4) * 2048
        for strip in range(2):
            rr = strip * 32
            nc.tensor.matmul(
                zps[rr:rr + 1, :], consts[jp:jp + 1, 289:290],
                zwin_t[jp:jp + 1,
                       jb + d * 1024 + strip * 512:jb + d * 1024 + (strip + 1) * 512],
                start=True, stop=False, tile_position=(jp, rr))
        for ri, (hcol, part) in enumerate(
                [(0, 0), (1, 0), (2, 0), (3, 0), (0, 1), (1, 1)]):
            kc = hcol % 2
            hsl = hstate[:, st8 + hcol:st8 + hcol + 1]
            for strip in range(2):
                rr = strip * 32
                nc.tensor.matmul(zps[rr:rr + 1, :], hsl, wt(part, kc, d, strip),
                                 start=False, stop=(ri == 5),
                                 tile_position=(0, rr))

    def lstm_post(d, gpar, t_glob):
        np8 = (1 - gpar) * 8 + d * 4
        zps = zpst[d][gpar]
        sg, cs, ig, fc, sc, hh = sig[d], cst[d], igt[d], fct[d], sct[d], hht[d]
        nc.scalar.activation(r2(sg, slice(0, 512)), zps[0:33, :], AF.Sigmoid)
        nc.vector.scalar_tensor_tensor(
            out=r2(ig, slice(0, 128)), in0=r2(sg, slice(256, 384)), scalar=-0.5,
            in1=r2(sg, slice(0, 128)), op0=OP.add, op1=OP.mult)
        nc.vector.tensor_tensor(
            out=r2(fc, slice(0, 128)), in0=r2(sg, slice(128, 256)),
            in1=r2(cs, slice(0, 128)), op=OP.mult)
        nc.vector.scalar_tensor_tensor(
            out=r2(cs, slice(0, 128)), in0=r2(ig, slice(0, 128)), scalar=2.0,
            in1=r2(fc, slice(0, 128)), op0=OP.mult, op1=OP.add)
        nc.scalar.activation(r2(sc, slice(0, 128)), r2(cs, slice(0, 128)),
                             AF.Sigmoid, scale=2.0)
        nc.vector.scalar_tensor_tensor(
            out=r2(hh, slice(0, 128)), in0=r2(sc, slice(0, 128)), scalar=-0.5,
            in1=r2(sg, slice(384, 512)), op0=OP.add, op1=OP.mult)
        tps = ps_small(f"tpsh{d}")
        nc.tensor.matmul(tps[0:128, 0:2], r2(hh, slice(0, 128)), I2c,
                         start=True, stop=True)
        nc.vector.tensor_copy(hstate[:, np8:np8 + 2], tps[0:128, 0:2])
        nc.vector.tensor_tensor(out=hstate[:, np8 + 2:np8 + 4], in0=tps[0:128, 0:2],
                                in1=hstate[:, np8:np8 + 2], op=OP.subtract)
        nc.sync.dma_start(hs[:, 8 * t_glob + 4 * d: 8 * t_glob + 4 * d + 4],
                          hstate[:, np8:np8 + 4])

    for w in range(NW):
        zwF = scr.tile([97, (WIN // 4) * 2048], fp32, name="zwF", tag="zw", bufs=5)
        zwB = scr.tile([97, (WIN // 4) * 2048], fp32, name="zwB", tag="zw", bufs=5)
        nc.sync.dma_start(
            zwF[0:97:32, :].rearrange("p (b c) -> p b c", b=WIN // 4),
            io["zx"][w * WIN:(w + 1) * WIN, :].rearrange(
                "(b p4) c -> p4 b c", p4=4))
        nc.sync.dma_start(
            zwB[0:97:32, :].rearrange("p (b c) -> p b c", b=WIN // 4),
            io["zx"][S - (w + 1) * WIN:S - w * WIN, :].rearrange(
                "(b p4) c -> p4 b c", p4=4))
        for j in range(WIN):
            g = w * WIN + j
            lstm_mm(0, j, g % 2, zwF[:])
            lstm_post(0, g % 2, g)
            lstm_mm(1, WIN - 1 - j, g % 2, zwB[:])
            lstm_post(1, g % 2, S - 1 - g)

    tc.strict_bb_all_engine_barrier()

    # ---------------- Stage C ----------------
    feats = scr.tile([48, S], fp32, name="feats", tag="sc8a")
    hsv = hs[:].rearrange("p (t c) -> p c t", c=8)
    WOh = WOT_hi_s[:].rearrange("p (c t) -> p c t", c=4)
    WOl = WOT_lo_s[:].rearrange("p (c t) -> p c t", c=4)
    HWID = min(1024, S)
    NH = S // HWID
    NBW = min(512, S)
    hi_cols = [0, 1, 4, 5]
    lo_cols = [2, 3, 6, 7]
    for hh_ in range(NH):
        fps = ps_big("fpsum")
        NB = HWID // NBW
        for ci in range(4):
            for part in range(3):
                hc = hi_cols[ci] if part in (0, 2) else lo_cols[ci]
                wv = WOh if part in (0, 1) else WOl
                for nb in range(NB):
                    tb = hh_ * HWID + nb * NBW
                    nc.tensor.matmul(
                        fps[0:48, nb * NBW:(nb + 1) * NBW], wv[:, ci, :],
                        hsv[:, hc, tb:tb + NBW],
                        start=(ci == 0 and part == 0),
                        stop=(ci == 3 and part == 2))
        nc.vector.tensor_copy(feats[:, hh_ * HWID:(hh_ + 1) * HWID],
                              fps[0:48, 0:HWID])

    # Export feats: exact full-length Viterbi runs in a second launch on the
    # host-stitched feats (chunked alphas can't replicate reference fp32
    # tie-breaks at long-range score bubbles; exact prefix alphas can).
    nc.sync.dma_start(io["feats_out"], feats[:])

    scr.release()
    psz.release()
    psum.release()
    persist.release()


def emit_de(nc, tc, io, S):
    """Stages D+E only (exact full-length Viterbi + backtrack) on feats input."""
    import concourse.mybir as mybir
    fp32 = mybir.dt.float32
    u16 = mybir.dt.uint16
    OP = mybir.AluOpType

    persist = tc.alloc_tile_pool(name="persist", bufs=1)
    pt = lambda shape, dt, name: persist.tile(shape, dt, name=name)
    consts = pt([128, 512], fp32, "consts")
    mx = pt([48, 8], fp32, "mx")
    bps = pt([48, S + 8], u16, "bps")
    tmp = pt([48, 48], fp32, "tmp")
    tmp2 = pt([48, 48], fp32, "tmp2")
    Mscr = pt([48, 4 * 48], fp32, "Mscr")
    lastrow = pt([1, 48], fp32, "lastrow")
    lmax = pt([1, 8], fp32, "lmax")
    lidx = pt([1, 8], mybir.dt.uint32, "lidx")
    lidxf = pt([1, 1], fp32, "lidxf")
    vrow = pt([1, 48], fp32, "vrow")
    trans_s = pt([48, 48], fp32, "trans_s")
    feats = pt([48, S], fp32, "feats")

    I48 = consts[0:48, 2:50]
    one_1 = consts[0:1, 50:51]
    iota_row = consts[0:1, 178:226]
    iota_col = consts[0:48, 226:227]
    iotaM = consts[0:48, 240:288]

    scr = tc.alloc_tile_pool(name="scratch", bufs=1)
    psum = tc.alloc_tile_pool(name="psum", bufs=1, space="PSUM")

    def ps_big(name):
        return psum.tile([128, 1024], fp32, name=name, tag="big", bufs=1)

    def ps_small(name):
        return psum.tile([128, 128], fp32, name=name, tag="small", bufs=2)

    nc.sync.dma_start(consts[:], io["consts"])
    nc.sync.dma_start(trans_s[:], io["trans"])
    nc.sync.dma_start(feats[:], io["feats_in"])

    tc.strict_bb_all_engine_barrier()

    HWID = min(1024, S)
    NH = S // HWID
    NBW = min(512, S)

    # ---------------- Stage D ----------------
    prev = feats[:, 0:1]
    for t in range(1, S):
        nc.vector.tensor_scalar(out=tmp[:], in0=trans_s[:], scalar1=prev,
                                scalar2=None, op0=OP.add)
        tpsv = ps_small("tpsv")
        nc.tensor.transpose(tpsv[0:48, 0:48], tmp[:], I48)
        nc.vector.tensor_scalar(out=tmp2[:], in0=tpsv[0:48, 0:48],
                                scalar1=feats[:, t:t + 1],
                                scalar2=None, op0=OP.add)
        nc.vector.max(mx[:], tmp2[:])
        nc.vector.max_index(bps[:, t - 1:t + 7], mx[:], tmp2[:])
        prev = mx[:, 0:1]

    Vmat = scr.tile([48, S], fp32, name="Vmat", tag="sc8b")
    bps_f = scr.tile([48, S - 1], fp32, name="bps_f", tag="sc8c")
    lps = ps_small("lpsum")
    nc.tensor.matmul(lps[0:1, 0:48], mx[:, 0:1], I48, start=True, stop=True)
    nc.vector.tensor_copy(lastrow[:], lps[0:1, 0:48])
    nc.vector.max(lmax[:], lastrow[:])
    nc.vector.max_index(lidx[:], lmax[:], lastrow[:])
    nc.vector.tensor_copy(lidxf[:], lidx[:, 0:1])
    nc.vector.tensor_scalar(out=vrow[:], in0=iota_row, scalar1=lidxf[0:1, 0:1],
                            scalar2=None, op0=OP.is_equal)
    vcol = ps_small("vcol")
    nc.tensor.matmul(vcol[0:48, 0:1], vrow[:], one_1, start=True, stop=True)
    nc.vector.tensor_copy(Vmat[:, S - 1:S], vcol[0:48, 0:1])

    nc.vector.tensor_copy(bps_f[:], bps[:, 0:S - 1])

    tc.strict_bb_all_engine_barrier()

    # ---------------- Stage E ----------------
    for u in range(S - 1):
        t = S - 2 - u
        ms = Mscr[:, (u % 4) * 48:(u % 4) * 48 + 48]
        nc.vector.tensor_scalar(out=ms, in0=iotaM, scalar1=bps_f[:, t:t + 1],
                                scalar2=None, op0=OP.is_equal)
        vp = ps_small("vp")
        nc.tensor.matmul(vp[0:48, 0:1], ms, Vmat[:, t + 1:t + 2],
                         start=True, stop=True)
        nc.vector.tensor_copy(Vmat[:, t:t + 1], vp[0:48, 0:1])

    pathi = scr.tile([1, S], mybir.dt.int32, name="pathi", tag="sc8d")
    for hh_ in range(NH):
        pps = ps_big("ppsum")
        for nb in range(HWID // NBW):
            tb = hh_ * HWID + nb * NBW
            nc.tensor.matmul(pps[0:1, nb * NBW:(nb + 1) * NBW], iota_col,
                             Vmat[:, tb:tb + NBW], start=True, stop=True)
        nc.vector.tensor_copy(pathi[:, hh_ * HWID:(hh_ + 1) * HWID],
                              pps[0:1, 0:HWID])
    nc.sync.dma_start(io["out_path"], pathi[:])

    scr.release()
    psum.release()
    persist.release()


def emit_d2(nc, tc, io, S):
    """Viterbi forward only (core 0): per step = tensor_scalar add (t1 =
    trans + prev_col) -> PE transpose -> tensor_tensor_reduce (t2 = t1T +
    obs, alpha = rowmax) with max_index in the transpose shadow.
    Bit-exact with reference fp32 op order."""
    import concourse.mybir as mybir
    fp32 = mybir.dt.float32
    u16 = mybir.dt.uint16
    OP = mybir.AluOpType

    persist = tc.alloc_tile_pool(name="persist", bufs=1)
    pt = lambda shape, dt, name: persist.tile(shape, dt, name=name)
    consts = pt([128, 512], fp32, "consts")
    trans_s = pt([48, 48], fp32, "trans_s")
    feats = pt([48, S], fp32, "feats")
    alpha = pt([48, S], fp32, "alpha")
    bps = pt([48, 8 * (S - 1)], u16, "bps")
    I48 = consts[0:48, 2:50]

    scr = tc.alloc_tile_pool(name="scratch", bufs=1)
    psum = tc.alloc_tile_pool(name="psum", bufs=1, space="PSUM")

    nc.sync.dma_start(consts[:], io["consts"])
    nc.sync.dma_start(trans_s[:], io["trans"])
    nc.sync.dma_start(feats[:], io["feats_in"])
    nc.vector.memset(alpha[:], 0.0)

    tc.strict_bb_all_engine_barrier()

    nc.vector.tensor_copy(alpha[:, 0:1], feats[:, 0:1])
    hist = []
    for t in range(1, S):
        t1 = scr.tile([48, 48], fp32, name="t1", tag="t1", bufs=2)
        t2 = scr.tile([48, 48], fp32, name="t2", tag="t2", bufs=3)
        mx = scr.tile([48, 8], fp32, name="mx", tag="mx", bufs=3)
        tps = psum.tile([48, 48], fp32, name="tps", tag="tps", bufs=2)
        prev = alpha[:, 0:1] if t == 1 else hist[-1][0][:, 0:1]
        nc.vector.tensor_scalar(out=t1[:], in0=trans_s[:],
                                scalar1=prev, scalar2=None, op0=OP.add)
        if t > 1:
            # previous step's backpointer extraction hides in this step's
            # transpose window (DVE FIFO: after ts1, before next max8)
            pmx, pt2 = hist[-1]
            nc.vector.max_index(bps[:, 8 * (t - 2):8 * (t - 1)],
                                pmx[:], pt2[:])
        nc.tensor.transpose(tps[:], t1[:], I48)
        nc.vector.tensor_scalar(out=t2[:], in0=tps[:],
                                scalar1=feats[:, t:t + 1], scalar2=None,
                                op0=OP.add)
        nc.vector.max(mx[:], t2[:])
        if t == S - 1:
            nc.vector.tensor_copy(alpha[:, t:t + 1], mx[:, 0:1])
        hist.append((mx, t2))
    pmx, pt2 = hist[-1]
    nc.vector.max_index(bps[:, 8 * (S - 2):8 * (S - 1)], pmx[:], pt2[:])

    nc.sync.dma_start(io["alpha_out"], alpha[:])
    nc.sync.dma_start(io["bps_out"], bps[:])
    scr.release()
    psum.release()
    persist.release()


def emit_e2(nc, tc, io, L):
    """Backtrack chunk compose (SPMD x8): each core composes its chunk's
    48-entry backpointer maps (one-hot matmul chain) and logs the full
    trajectory P[u, e] = path state at local position u given entry e."""
    import concourse.mybir as mybir
    fp32 = mybir.dt.float32
    u16 = mybir.dt.uint16
    OP = mybir.AluOpType

    persist = tc.alloc_tile_pool(name="persist", bufs=1)
    pt = lambda shape, dt, name: persist.tile(shape, dt, name=name)
    consts = pt([128, 512], fp32, "consts")
    bpsu = pt([48, L], u16, "bpsu")
    bpf = pt([48, L], fp32, "bpf")
    msall = pt([48, 48 * L], fp32, "msall")
    P = pt([1, 48 * L], fp32, "P")
    Vs = [pt([48, 48], fp32, "V0"), pt([48, 48], fp32, "V1")]
    I48 = consts[0:48, 2:50]
    iotaM = consts[0:48, 240:288]
    iota_col = consts[0:48, 226:227]

    psum = tc.alloc_tile_pool(name="psum", bufs=1, space="PSUM")

    nc.sync.dma_start(consts[:], io["consts"])
    nc.sync.dma_start(bpsu[:], io["bps_in"])
    tc.strict_bb_all_engine_barrier()

    nc.vector.tensor_copy(bpf[:], bpsu[:])
    # precompute all one-hot ms matrices (no serial deps)
    for u in range(L):
        nc.vector.tensor_scalar(out=msall[:, 48 * u:48 * u + 48], in0=iotaM,
                                scalar1=bpf[:, u:u + 1], scalar2=None,
                                op0=OP.is_equal)
    # serial compose from the top; V_{L} = I48
    for k in range(L):
        u = L - 1 - k
        ms = msall[:, 48 * u:48 * u + 48]
        rhs = I48 if k == 0 else Vs[(k - 1) % 2][:]
        vps = psum.tile([48, 48], fp32, name="vps", tag="vps", bufs=2)
        sps = psum.tile([1, 48], fp32, name="sps", tag="sps", bufs=2)
        nc.tensor.matmul(vps[:], ms, rhs, start=True, stop=True)
        nc.vector.tensor_copy(Vs[k % 2][:], vps[:])
        nc.tensor.matmul(sps[:], iota_col, Vs[k % 2][:], start=True, stop=True)
        nc.vector.tensor_copy(P[:, 48 * u:48 * u + 48], sps[:])

    nc.sync.dma_start(io["p_out"], P[:])
    psum.release()
    persist.release()


def declare_io(nc, S, stages="ABC"):
    import concourse.mybir as mybir
    io = {}
    if "A" in stages:
        specs = [
            ("sent", (128, S // 128), mybir.dt.int32, "ExternalInput"),
            ("embed", (V, E), mybir.dt.float32, "ExternalInput"),
            ("WT_hi", (128, 2, 2048), mybir.dt.bfloat16, "ExternalInput"),
            ("WT_lo", (128, 2, 2048), mybir.dt.bfloat16, "ExternalInput"),
            ("WIT_hi", (128, 2, 2048), mybir.dt.bfloat16, "ExternalInput"),
            ("WIT_lo", (128, 2, 2048), mybir.dt.bfloat16, "ExternalInput"),
            ("bias_row", (1, 2048), mybir.dt.float32, "ExternalInput"),
            ("WOT_hi", (128, 4, 48), mybir.dt.bfloat16, "ExternalInput"),
            ("WOT_lo", (128, 4, 48), mybir.dt.bfloat16, "ExternalInput"),
            ("trans", (48, 48), mybir.dt.float32, "ExternalInput"),
            ("h0_init", (128, 8), mybir.dt.bfloat16, "ExternalInput"),
            ("c0_init", (4, 128), mybir.dt.float32, "ExternalInput"),
            ("consts", (128, 512), mybir.dt.float32, "ExternalInput"),
            ("zx", (S, 2048), mybir.dt.float32, "Internal"),
            ("feats_out", (48, S), mybir.dt.float32, "ExternalOutput"),
        ]
    elif stages == "D2":
        specs = [
            ("consts", (128, 512), mybir.dt.float32, "ExternalInput"),
            ("trans", (48, 48), mybir.dt.float32, "ExternalInput"),
            ("feats_in", (48, S), mybir.dt.float32, "ExternalInput"),
            ("alpha_out", (48, S), mybir.dt.float32, "ExternalOutput"),
            ("bps_out", (48, 8 * (S - 1)), mybir.dt.uint16, "ExternalOutput"),
        ]
    elif stages == "E2":
        L = S  # reuse S slot as chunk length
        specs = [
            ("consts", (128, 512), mybir.dt.float32, "ExternalInput"),
            ("bps_in", (48, L), mybir.dt.uint16, "ExternalInput"),
            ("p_out", (1, 48 * L), mybir.dt.float32, "ExternalOutput"),
        ]
    else:
        specs = [
            ("consts", (128, 512), mybir.dt.float32, "ExternalInput"),
            ("trans", (48, 48), mybir.dt.float32, "ExternalInput"),
            ("feats_in", (48, S), mybir.dt.float32, "ExternalInput"),
            ("out_path", (1, S), mybir.dt.int32, "ExternalOutput"),
        ]
    for nm, shp, dt, kind in specs:
        io[nm] = nc.dram_tensor(nm, shp, dt, kind=kind).ap()
    return io


def build_nc(S, WIN, stages="ABC", debug=False):
    import concourse.bacc as bacc
    from concourse import tile
    nc = bacc.Bacc("TRN2", target_bir_lowering=False, debug=debug)
    io = declare_io(nc, S, stages)
    with tile.TileContext(nc) as tc:
        if "A" in stages:
            emit(nc, tc, io, S, WIN)
        elif stages == "D2":
            emit_d2(nc, tc, io, S)
        elif stages == "E2":
            emit_e2(nc, tc, io, S)
        else:
            emit_de(nc, tc, io, S)
    nc.compile()
    return nc


LAST_EXEC_NS = None
LAST_EXEC_PARTS = None


def _ensure_ntff_hook():
    """This image's antenv lacks axon_hooks; synthesize it and install the
    ctypes-based NTFF profile hook from trn_agent_boot so trace=True works."""
    try:
        from antenv.axon_hooks import get_axon_ntff_profile_hook
        if get_axon_ntff_profile_hook() is not None:
            return True
    except ImportError:
        import sys as _s, types as _t
        try:
            import antenv
        except ImportError:
            return False
        mod = _t.ModuleType("antenv.axon_hooks")
        mod._hook = None
        mod.set_axon_ntff_profile_hook = lambda h: setattr(mod, "_hook", h)
        mod.get_axon_ntff_profile_hook = lambda: mod._hook
        _s.modules["antenv.axon_hooks"] = mod
        antenv.axon_hooks = mod
    try:
        from antenv.axon_hooks import (get_axon_ntff_profile_hook,
                                       set_axon_ntff_profile_hook)
        if get_axon_ntff_profile_hook() is None:
            from trn_agent_boot.trn_boot import _ntff_profile_via_ctypes
            set_axon_ntff_profile_hook(
                _ntff_profile_via_ctypes('/opt/axon/libaxon_pjrt.so'))
        return get_axon_ntff_profile_hook() is not None
    except Exception:
        return False


_SPMD_CALLS = 0


def _run_spmd(nc, in_maps, core_ids):
    """run_bass_kernel_spmd with trace (for exec_time_ns); falls back to
    untraced on any profiling failure."""
    import os
    global _SPMD_CALLS
    _SPMD_CALLS += 1
    from concourse import bass_utils
    tmp = os.environ.get("KTRACE_DIR")
    if tmp:
        tmp = os.path.join(tmp, f"l{_SPMD_CALLS}")
        os.makedirs(tmp, exist_ok=True)
    try:
        if not _ensure_ntff_hook():
            raise RuntimeError("no ntff hook")
        return bass_utils.run_bass_kernel_spmd(
            nc, in_maps, core_ids=core_ids, trace=True,
            tmpdir=(tmp if tmp else None))
    except Exception as e:
        print(f"traced run failed ({type(e).__name__}: {e}); retrying untraced")
        return bass_utils.run_bass_kernel_spmd(nc, in_maps, core_ids=core_ids)


def kernel(**inputs):
    """Chunked-SPMD BiLSTM-CRF in two launches.

    Launch 1: each of the 8 cores runs the BiLSTM + emission projection for
    one 256-step slice using a 384-step window (the LSTM forget gate makes a
    44-100 step warmup converge to the full-sequence hidden state to ~2e-7);
    the host stitches the per-core feats into the full (48, 2048) matrix.

    Launch 2: exact full-length Viterbi decode + backtrack on the stitched
    feats (single core) - reproduces the reference's fp32 tie-breaking, which
    windowed alphas cannot."""
    global LAST_EXEC_NS
    from concourse import bass_utils
    Sfull = int(np.asarray(inputs["sent"]).shape[0])
    NC = 8
    L = Sfull // NC          # 256 output steps per core
    SW_ = 384                # window length per core
    shared = prep_shared(inputs)
    SHIFT = 84               # fwd warmup 84, bwd warmup 44 steps
    in_maps, offs = [], []
    for c in range(NC):
        ws = min(max(c * L - SHIFT, 0), Sfull - SW_)
        dev = dict(shared)
        dev.update(prep_core(inputs, ws, SW_,
                             fwd_real=(ws == 0),
                             bwd_real=(ws + SW_ == Sfull)))
        in_maps.append(dev)
        offs.append(c * L - ws)
    nc1 = build_nc(SW_, WIN=4, stages="ABC")
    res1 = _run_spmd(nc1, in_maps, core_ids=list(range(NC)))
    feats = np.zeros((48, Sfull), np.float32)
    for c in range(NC):
        fw = np.asarray(res1.results[c]["feats_out"], np.float32)
        feats[:, c * L:(c + 1) * L] = fw[:, offs[c]:offs[c] + L]

    global LAST_EXEC_PARTS
    try:
        # Launch 2: Viterbi forward on core 0 (alphas + raw backpointers).
        nc2 = build_nc(Sfull, WIN=4, stages="D2")
        dev2 = {"consts": shared["consts"], "trans": shared["trans"],
                "feats_in": np.ascontiguousarray(feats)}
        res2 = _run_spmd(nc2, [dev2], core_ids=[0])
        alpha = np.asarray(res2.results[0]["alpha_out"], np.float32)
        bps8 = np.asarray(res2.results[0]["bps_out"])    # (48, 8*(S-1)) u16
        bp = bps8.reshape(48, Sfull - 1, 8)[:, :, 0]     # ref_bps[i] = bp[:, i]

        # Launch 3: 8-core parallel backtrack compose (48-entry trajectories).
        L = 256
        starts = [c * L for c in range(7)] + [Sfull - 1 - L]
        nc3 = build_nc(L, WIN=4, stages="E2")
        in3 = [{"consts": shared["consts"],
                "bps_in": np.ascontiguousarray(bp[:, s:s + L].astype(np.uint16))}
               for s in starts]
        res3 = _run_spmd(nc3, in3, core_ids=list(range(8)))

        last = int(np.argmax(alpha[:, -1]))
        path = np.zeros(Sfull, np.int64)
        path[Sfull - 1] = last
        for c in range(7, -1, -1):
            s0 = starts[c]
            e = last if c == 7 else int(path[s0 + L])
            P = np.asarray(res3.results[c]["p_out"], np.float32).reshape(L, 48)
            path[s0:s0 + L] = P[:, e].astype(np.int64)
        t1 = res1.exec_time_ns
        t2 = res2.exec_time_ns
        t3 = res3.exec_time_ns
        LAST_EXEC_PARTS = (t1, t2, t3)
        parts = [t1, t2, t3]
        LAST_EXEC_NS = sum(parts) if all(p is not None for p in parts) else None
        return path.astype(np.int32)
    except Exception as e:
        print(f"fast viterbi path failed ({type(e).__name__}: {e}); "
              f"falling back to single-core DE")
        nc2 = build_nc(Sfull, WIN=4, stages="DE")
        dev2 = {"consts": shared["consts"], "trans": shared["trans"],
                "feats_in": np.ascontiguousarray(feats)}
        res2 = _run_spmd(nc2, [dev2], core_ids=[0])
        t1 = res1.exec_time_ns
        t2 = res2.exec_time_ns
        LAST_EXEC_PARTS = (t1, t2)
        LAST_EXEC_NS = (t1 + t2) if (t1 is not None and t2 is not None) else None
        return res2.results[0]["out_path"].reshape(Sfull).astype(np.int32)



# revision 21
# speedup vs baseline: 1.1199x; 1.1199x over previous
"""BiLSTM-CRF Viterbi decode on Trainium2 (Bass/Tile).

Single-core compute graph, run SPMD-redundantly on 8 NeuronCores (each core
computes the full result on identical inputs; core 0's output is returned).

Stages:
  A. embedding gather (indirect DMA) -> X ; PE-transpose -> X.T ; bf16 hi/lo split;
     zx = X @ Wih'.T + b'  (3-product hi/lo bf16 matmuls = ~fp32 precision)
     -> zx DRAM, gate-column-permuted, sigma-only prescaling folded in.
  B. two phase-shifted sequential LSTM recurrences (fwd rows {0,32}, bwd {64,96}):
     per step: zx round + 6 weight rounds (p3) into PSUM -> sigma(z) ->
     fused cell update (scalar_tensor_tensor) -> h/2 -> PE transpose ->
     bf16 hi/lo state for next step's stationary + hs log (partition-major).
  C. feats = hs @ (2*W_out).T (p3) -> (48, S) partition-major.
  D. Viterbi forward, exact reference op-order: tmp=trans+prev ; PE transpose ;
     +obs ; max8 + max_index backpointers.  (bit-exact vs jax fp32 on CPU)
  E. backtrack as one-hot matvec chain on PE (exact integers); path = iota @ V.

Launch structure (v2):
  launch 1 (8 cores):  A-C chunked BiLSTM -> feats, host-stitched.
  launch 2 (core 0):   D2 = Viterbi forward only; per step ts-add -> PE
     transpose -> ts-add(+obs) -> max8, with max_index hidden in the next
     step's transpose window; exports final alpha + all backpointers.
  launch 3 (8 cores):  E2 = per-chunk backtrack compose: one-hot matmul
     chain over 48x48 maps logging the full 48-entry trajectory P[u,e];
     host picks entry states at chunk boundaries and stitches the path.
  (falls back to the single-core DE program on any failure.)
"""

import numpy as np
import ml_dtypes

V, E, H, T = 100000, 256, 512, 48
HD = H // 2
G4 = 4 * HD

bf16 = ml_dtypes.bfloat16

# gate-column permutation: [i.lo f.lo g.lo o.lo | i.hi f.hi g.hi o.hi] (128 each)
PERM = np.r_[0:128, 256:384, 512:640, 768:896, 128:256, 384:512, 640:768, 896:1024]


def _split_hilo(x):
    hi = x.astype(bf16)
    lo = (x - hi.astype(np.float32)).astype(bf16)
    return hi, lo


def _pack_k(w_t):
    """(256, C) -> (128, 2, C) with [p, c, :] = w_t[c*128+p, :]"""
    return np.ascontiguousarray(w_t.reshape(2, 128, -1).transpose(1, 0, 2))


def prep_shared(inputs):
    """Weights/consts shared by all cores (window-independent)."""
    f32 = np.float32
    out = {"embed": np.ascontiguousarray(np.asarray(inputs["embed_table"], f32))}

    wt_hi, wt_lo, wit_hi, wit_lo, brow = [], [], [], [], []
    for d in ("f", "b"):
        Whh = np.asarray(inputs[f"Whh_{d}"], f32)
        Wih = np.asarray(inputs[f"Wih_{d}"], f32)
        b = np.asarray(inputs[f"b_{d}"], f32).copy()
        gscale = np.ones((G4, 1), f32)
        gscale[2 * HD:3 * HD] = 2.0          # sigma-form tanh: g rows x2
        Wrec = (2.0 * Whh) * gscale          # x2 again: state is h/2
        Wzx = Wih * gscale
        b2 = b * gscale[:, 0]
        hi, lo = _split_hilo(_pack_k(np.ascontiguousarray(Wrec.T[:, PERM])))
        wt_hi.append(hi); wt_lo.append(lo)
        hi, lo = _split_hilo(_pack_k(np.ascontiguousarray(Wzx.T[:, PERM])))
        wit_hi.append(hi); wit_lo.append(lo)
        brow.append(b2[PERM])
    out["WT_hi"] = np.ascontiguousarray(np.concatenate(wt_hi, -1))    # (128,2,2048)
    out["WT_lo"] = np.ascontiguousarray(np.concatenate(wt_lo, -1))
    out["WIT_hi"] = np.ascontiguousarray(np.concatenate(wit_hi, -1))
    out["WIT_lo"] = np.ascontiguousarray(np.concatenate(wit_lo, -1))
    out["bias_row"] = np.concatenate(brow)[None, :].astype(f32)       # (1, 2048)

    W_out2 = 2.0 * np.asarray(inputs["W_out"], f32)
    woh, wol = _split_hilo(
        np.ascontiguousarray(W_out2.T).reshape(4, 128, 48).transpose(1, 0, 2))
    out["WOT_hi"] = np.ascontiguousarray(woh)                          # (128,4,48)
    out["WOT_lo"] = np.ascontiguousarray(wol)

    out["trans"] = np.ascontiguousarray(np.asarray(inputs["transitions"], f32))

    # consts (128, 512):
    # 0:2 I2 at rows {0,32},{64,96}; 2:50 I48; 50:178 ones row0; 178:226 iota row0;
    # 226 iota col; 240:288 iotaM[k,i]=i; 320:448 I128
    consts = np.zeros((128, 512), f32)
    consts[0, 0] = 1.0
    consts[32, 1] = 1.0
    consts[0:48, 2:50] = np.eye(48, dtype=f32)
    consts[0, 50:178] = 1.0
    consts[0, 178:226] = np.arange(48, dtype=f32)
    consts[0:48, 226] = np.arange(48, dtype=f32)
    consts[0:48, 240:288] = np.arange(48, dtype=f32)[None, :]
    for r0 in (0, 32, 64, 96):
        consts[r0, 289] = 1.0
    consts[:, 320:448] = np.eye(128, dtype=f32)
    out["consts"] = consts
    return out


def prep_core(inputs, ws, S, fwd_real, bwd_real):
    """Per-core window inputs: sent slice + effective h0/c0 (zero for warmup
    chains, real initial state only at the true sequence edges)."""
    f32 = np.float32
    sent = np.asarray(inputs["sent"]).astype(np.int32)[ws:ws + S]
    h0 = np.asarray(inputs["h0"], f32).copy()
    c0 = np.asarray(inputs["c0"], f32).copy()
    if not fwd_real:
        h0[0] = 0.0; c0[0] = 0.0
    if not bwd_real:
        h0[1] = 0.0; c0[1] = 0.0
    h0 = h0 / 2.0
    out = {"sent": sent.reshape(S // 128, 128).T.copy()}  # [p, w] = sent[w*128+p]
    hblk = []
    for d in range(2):
        hi, lo = _split_hilo(np.ascontiguousarray(h0[d].reshape(2, 128).T))
        hblk.append(np.concatenate([hi, lo], 1))
    out["h0_init"] = np.ascontiguousarray(np.concatenate(hblk, 1))     # (128,8)
    out["c0_init"] = np.ascontiguousarray(
        np.concatenate([c0[0].reshape(2, 128), c0[1].reshape(2, 128)], 0))  # (4,128)
    return out


def emit(nc, tc, io, S, WIN, stages="ABCDE"):
    import concourse.bass as bass
    import concourse.mybir as mybir

    fp32 = mybir.dt.float32
    bf = mybir.dt.bfloat16
    u16 = mybir.dt.uint16
    AF = mybir.ActivationFunctionType
    OP = mybir.AluOpType

    assert S % 128 == 0 and WIN % 4 == 0 and S % WIN == 0
    NW = S // WIN
    SW = S // 128

    persist = tc.alloc_tile_pool(name="persist", bufs=1)
    pt = lambda shape, dt, name: persist.tile(shape, dt, name=name)

    consts = pt([128, 512], fp32, "consts")
    WT_hi = pt([128, 2 * 2048], bf, "WT_hi_s")
    WT_lo = pt([128, 2 * 2048], bf, "WT_lo_s")
    hs = pt([128, 8 * S], bf, "hs")
    hstate = pt([128, 16], bf, "hstate")
    mx = pt([48, 8], fp32, "mx")
    bps = pt([48, S + 8], u16, "bps")     # col t-1 <- idx0 of step t (overlapped writes)
    tmp = pt([48, 48], fp32, "tmp")
    tmp2 = pt([48, 48], fp32, "tmp2")
    Mscr = pt([48, 4 * 48], fp32, "Mscr")
    lastrow = pt([1, 48], fp32, "lastrow")
    lmax = pt([1, 8], fp32, "lmax")
    lidx = pt([1, 8], mybir.dt.uint32, "lidx")
    lidxf = pt([1, 1], fp32, "lidxf")
    vrow = pt([1, 48], fp32, "vrow")
    trans_s = pt([48, 48], fp32, "trans_s")
    WOT_hi_s = pt([128, 4 * 48], bf, "WOT_hi_s")
    WOT_lo_s = pt([128, 4 * 48], bf, "WOT_lo_s")

    # per-direction working tiles; rows used: {0, 32}
    sig = [pt([33, 512], fp32, "sigF"), pt([33, 512], fp32, "sigB")]
    cst = [pt([33, 128], fp32, "cstF"), pt([33, 128], fp32, "cstB")]
    igt = [pt([33, 128], fp32, "igF"), pt([33, 128], fp32, "igB")]
    fct = [pt([33, 128], fp32, "fcF"), pt([33, 128], fp32, "fcB")]
    sct = [pt([33, 128], fp32, "scF"), pt([33, 128], fp32, "scB")]
    hht = [pt([33, 128], fp32, "hhF"), pt([33, 128], fp32, "hhB")]

    I2c = consts[0:33, 0:2]
    I48 = consts[0:48, 2:50]
    ones_row = consts[0:1, 50:178]
    one_1 = consts[0:1, 50:51]
    iota_row = consts[0:1, 178:226]
    iota_col = consts[0:48, 226:227]
    iotaM = consts[0:48, 240:288]
    I128 = consts[:, 320:448]

    # shared scratch pools (single pools for the whole program; tags give WAR sync)
    scr = tc.alloc_tile_pool(name="scratch", bufs=1)
    psum = tc.alloc_tile_pool(name="psum", bufs=1, space="PSUM")

    psz = tc.alloc_tile_pool(name="psz", bufs=1, space="PSUM")
    zpst = [[psz.tile([128, 512], fp32, name=f"zper{d}{p}") for p in range(2)]
            for d in range(2)]

    def ps_big(name):    # 2 banks
        return psum.tile([128, 1024], fp32, name=name, tag="big", bufs=1)

    def ps_small(name):  # 1 bank x2
        return psum.tile([128, 128], fp32, name=name, tag="small", bufs=2)

    nc.sync.dma_start(consts[:], io["consts"])
    nc.sync.dma_start(WT_hi[:], io["WT_hi"].rearrange("p c g -> p (c g)"))
    nc.sync.dma_start(WT_lo[:], io["WT_lo"].rearrange("p c g -> p (c g)"))
    nc.sync.dma_start(hstate[:, 0:8], io["h0_init"])
    for d_ in range(2):
        for t_ in (sig[d_], cst[d_], igt[d_], fct[d_], sct[d_], hht[d_]):
            nc.vector.memset(t_[:], 0.0)
        for p_ in range(2):
            nc.vector.memset(zpst[d_][p_][:], 0.0)
    nc.sync.dma_start(cst[0][0:33:32, :], io["c0_init"][0:2, :])
    nc.sync.dma_start(cst[1][0:33:32, :], io["c0_init"][2:4, :])
    nc.sync.dma_start(trans_s[:], io["trans"])
    nc.sync.dma_start(WOT_hi_s[:], io["WOT_hi"].rearrange("p c t -> p (c t)"))
    nc.sync.dma_start(WOT_lo_s[:], io["WOT_lo"].rearrange("p c t -> p (c t)"))

    WTh = WT_hi[:].rearrange("p (c g) -> p c g", c=2)
    WTl = WT_lo[:].rearrange("p (c g) -> p c g", c=2)

    def wt(part, kc, d, strip):
        t = WTh if part == 0 else WTl
        base = d * 1024 + strip * 512
        return t[:, kc, base:base + 512]

    # ---------------- Stage A ----------------
    sentb = scr.tile([128, SW], mybir.dt.int32, name="sentb", tag="sentb")
    X = scr.tile([128, SW * 256], fp32, name="X", tag="gx")
    XT_hi = scr.tile([128, 2 * S], bf, name="XT_hi", tag="sc8a")
    XT_lo = scr.tile([128, 2 * S], bf, name="XT_lo", tag="sc8b")
    bias_s = scr.tile([1, 2048], fp32, name="bias_s", tag="bias")
    WIT_hi_s = scr.tile([128, 2 * 2048], bf, name="WIT_hi_s", tag="sc8c")
    WIT_lo_s = scr.tile([128, 2 * 2048], bf, name="WIT_lo_s", tag="sc8d")

    nc.sync.dma_start(bias_s[:], io["bias_row"])
    nc.sync.dma_start(sentb[:], io["sent"])
    nc.sync.dma_start(WIT_hi_s[:], io["WIT_hi"].rearrange("p c g -> p (c g)"))
    nc.sync.dma_start(WIT_lo_s[:], io["WIT_lo"].rearrange("p c g -> p (c g)"))

    Xv = X[:].rearrange("p (w e) -> p w e", w=SW)
    for w in range(SW):
        nc.gpsimd.indirect_dma_start(
            out=Xv[:, w, :], out_offset=None,
            in_=io["embed"],
            in_offset=bass.IndirectOffsetOnAxis(ap=sentb[:, w:w + 1], axis=0))

    XTh = XT_hi[:].rearrange("p (c t) -> p c t", c=2)
    XTl = XT_lo[:].rearrange("p (c t) -> p c t", c=2)
    for w in range(SW):
        for ec in range(2):
            tps = ps_small("tpsa")
            nc.tensor.transpose(tps[:], Xv[:, w, ec * 128:(ec + 1) * 128], I128)
            nc.vector.tensor_copy(XTh[:, ec, w * 128:(w + 1) * 128], tps[:])
            nc.vector.tensor_tensor(out=XTl[:, ec, w * 128:(w + 1) * 128],
                                    in0=tps[:],
                                    in1=XTh[:, ec, w * 128:(w + 1) * 128],
                                    op=OP.subtract)

    WITh = WIT_hi_s[:].rearrange("p (c g) -> p c g", c=2)
    WITl = WIT_lo_s[:].rearrange("p (c g) -> p c g", c=2)

    for tt in range(SW):
        for gh in range(2):
            zps = ps_big("zxps")
            zsb = scr.tile([128, 1024], fp32, name="zxsb", tag="sc8e", bufs=2)
            for nb in range(2):
                cb = gh * 1024 + nb * 512
                nc.tensor.matmul(zps[:, nb * 512:(nb + 1) * 512], ones_row,
                                 bias_s[0:1, cb:cb + 512],
                                 start=True, stop=False)
            for ri, (Xp, Wp) in enumerate([(XTh, WITh), (XTl, WITh), (XTh, WITl)]):
                for kc in range(2):
                    for nb in range(2):
                        cb = gh * 1024 + nb * 512
                        nc.tensor.matmul(
                            zps[:, nb * 512:(nb + 1) * 512],
                            Xp[:, kc, tt * 128:(tt + 1) * 128],
                            Wp[:, kc, cb:cb + 512],
                            start=False, stop=(ri == 2 and kc == 1))
            nc.vector.tensor_copy(zsb[:], zps[:])
            nc.sync.dma_start(io["zx"][tt * 128:(tt + 1) * 128, gh * 1024:(gh + 1) * 1024],
                              zsb[:])

    tc.strict_bb_all_engine_barrier()

    # ---------------- Stage B ----------------
    def r2(tile_, cols):
        return tile_[0:33, cols]

    def lstm_mm(d, j, gpar, zwin_t):
        st8 = gpar * 8 + d * 4
        zps = zpst[d][gpar]
        jp = 32 * (j % 4)
        jb = (j // 4) * 2048
        for strip in range(2):
            rr = strip * 32
            nc.tensor.matmul(
                zps[rr:rr + 1, :], consts[jp:jp + 1, 289:290],
                zwin_t[jp:jp + 1,
                       jb + d * 1024 + strip * 512:jb + d * 1024 + (strip + 1) * 512],
                start=True, stop=False, tile_position=(jp, rr))
        for ri, (hcol, part) in enumerate(
                [(0, 0), (1, 0), (2, 0), (3, 0), (0, 1), (1, 1)]):
            kc = hcol % 2
            hsl = hstate[:, st8 + hcol:st8 + hcol + 1]
            for strip in range(2):
                rr = strip * 32
                nc.tensor.matmul(zps[rr:rr + 1, :], hsl, wt(part, kc, d, strip),
                                 start=False, stop=(ri == 5),
                                 tile_position=(0, rr))

    def lstm_post(d, gpar, t_glob):
        np8 = (1 - gpar) * 8 + d * 4
        zps = zpst[d][gpar]
        sg, cs, ig, fc, sc, hh = sig[d], cst[d], igt[d], fct[d], sct[d], hht[d]
        nc.scalar.activation(r2(sg, slice(0, 512)), zps[0:33, :], AF.Sigmoid)
        nc.vector.scalar_tensor_tensor(
            out=r2(ig, slice(0, 128)), in0=r2(sg, slice(256, 384)), scalar=-0.5,
            in1=r2(sg, slice(0, 128)), op0=OP.add, op1=OP.mult)
        nc.vector.tensor_tensor(
            out=r2(fc, slice(0, 128)), in0=r2(sg, slice(128, 256)),
            in1=r2(cs, slice(0, 128)), op=OP.mult)
        nc.vector.scalar_tensor_tensor(
            out=r2(cs, slice(0, 128)), in0=r2(ig, slice(0, 128)), scalar=2.0,
            in1=r2(fc, slice(0, 128)), op0=OP.mult, op1=OP.add)
        nc.scalar.activation(r2(sc, slice(0, 128)), r2(cs, slice(0, 128)),
                             AF.Sigmoid, scale=2.0)
        nc.vector.scalar_tensor_tensor(
            out=r2(hh, slice(0, 128)), in0=r2(sc, slice(0, 128)), scalar=-0.5,
            in1=r2(sg, slice(384, 512)), op0=OP.add, op1=OP.mult)
        tps = ps_small(f"tpsh{d}")
        nc.tensor.matmul(tps[0:128, 0:2], r2(hh, slice(0, 128)), I2c,
                         start=True, stop=True)
        nc.vector.tensor_copy(hstate[:, np8:np8 + 2], tps[0:128, 0:2])
        nc.vector.tensor_tensor(out=hstate[:, np8 + 2:np8 + 4], in0=tps[0:128, 0:2],
                                in1=hstate[:, np8:np8 + 2], op=OP.subtract)
        nc.sync.dma_start(hs[:, 8 * t_glob + 4 * d: 8 * t_glob + 4 * d + 4],
                          hstate[:, np8:np8 + 4])

    for w in range(NW):
        zwF = scr.tile([97, (WIN // 4) * 2048], fp32, name="zwF", tag="zw", bufs=5)
        zwB = scr.tile([97, (WIN // 4) * 2048], fp32, name="zwB", tag="zw", bufs=5)
        nc.sync.dma_start(
            zwF[0:97:32, :].rearrange("p (b c) -> p b c", b=WIN // 4),
            io["zx"][w * WIN:(w + 1) * WIN, :].rearrange(
                "(b p4) c -> p4 b c", p4=4))
        nc.sync.dma_start(
            zwB[0:97:32, :].rearrange("p (b c) -> p b c", b=WIN // 4),
            io["zx"][S - (w + 1) * WIN:S - w * WIN, :].rearrange(
                "(b p4) c -> p4 b c", p4=4))
        for j in range(WIN):
            g = w * WIN + j
            lstm_mm(0, j, g % 2, zwF[:])
            lstm_mm(1, WIN - 1 - j, g % 2, zwB[:])
            lstm_post(0, g % 2, g)
            lstm_post(1, g % 2, S - 1 - g)

    tc.strict_bb_all_engine_barrier()

    # ---------------- Stage C ----------------
    feats = scr.tile([48, S], fp32, name="feats", tag="sc8a")
    hsv = hs[:].rearrange("p (t c) -> p c t", c=8)
    WOh = WOT_hi_s[:].rearrange("p (c t) -> p c t", c=4)
    WOl = WOT_lo_s[:].rearrange("p (c t) -> p c t", c=4)
    HWID = min(1024, S)
    NH = S // HWID
    NBW = min(512, S)
    hi_cols = [0, 1, 4, 5]
    lo_cols = [2, 3, 6, 7]
    for hh_ in range(NH):
        fps = ps_big("fpsum")
        NB = HWID // NBW
        for ci in range(4):
            for part in range(3):
                hc = hi_cols[ci] if part in (0, 2) else lo_cols[ci]
                wv = WOh if part in (0, 1) else WOl
                for nb in range(NB):
                    tb = hh_ * HWID + nb * NBW
                    nc.tensor.matmul(
                        fps[0:48, nb * NBW:(nb + 1) * NBW], wv[:, ci, :],
                        hsv[:, hc, tb:tb + NBW],
                        start=(ci == 0 and part == 0),
                        stop=(ci == 3 and part == 2))
        nc.vector.tensor_copy(feats[:, hh_ * HWID:(hh_ + 1) * HWID],
                              fps[0:48, 0:HWID])

    # Export feats: exact full-length Viterbi runs in a second launch on the
    # host-stitched feats (chunked alphas can't replicate reference fp32
    # tie-breaks at long-range score bubbles; exact prefix alphas can).
    nc.sync.dma_start(io["feats_out"], feats[:])

    scr.release()
    psz.release()
    psum.release()
    persist.release()


def emit_de(nc, tc, io, S):
    """Stages D+E only (exact full-length Viterbi + backtrack) on feats input."""
    import concourse.mybir as mybir
    fp32 = mybir.dt.float32
    u16 = mybir.dt.uint16
    OP = mybir.AluOpType

    persist = tc.alloc_tile_pool(name="persist", bufs=1)
    pt = lambda shape, dt, name: persist.tile(shape, dt, name=name)
    consts = pt([128, 512], fp32, "consts")
    mx = pt([48, 8], fp32, "mx")
    bps = pt([48, S + 8], u16, "bps")
    tmp = pt([48, 48], fp32, "tmp")
    tmp2 = pt([48, 48], fp32, "tmp2")
    Mscr = pt([48, 4 * 48], fp32, "Mscr")
    lastrow = pt([1, 48], fp32, "lastrow")
    lmax = pt([1, 8], fp32, "lmax")
    lidx = pt([1, 8], mybir.dt.uint32, "lidx")
    lidxf = pt([1, 1], fp32, "lidxf")
    vrow = pt([1, 48], fp32, "vrow")
    trans_s = pt([48, 48], fp32, "trans_s")
    feats = pt([48, S], fp32, "feats")

    I48 = consts[0:48, 2:50]
    one_1 = consts[0:1, 50:51]
    iota_row = consts[0:1, 178:226]
    iota_col = consts[0:48, 226:227]
    iotaM = consts[0:48, 240:288]

    scr = tc.alloc_tile_pool(name="scratch", bufs=1)
    psum = tc.alloc_tile_pool(name="psum", bufs=1, space="PSUM")

    def ps_big(name):
        return psum.tile([128, 1024], fp32, name=name, tag="big", bufs=1)

    def ps_small(name):
        return psum.tile([128, 128], fp32, name=name, tag="small", bufs=2)

    nc.sync.dma_start(consts[:], io["consts"])
    nc.sync.dma_start(trans_s[:], io["trans"])
    nc.sync.dma_start(feats[:], io["feats_in"])

    tc.strict_bb_all_engine_barrier()

    HWID = min(1024, S)
    NH = S // HWID
    NBW = min(512, S)

    # ---------------- Stage D ----------------
    prev = feats[:, 0:1]
    for t in range(1, S):
        nc.vector.tensor_scalar(out=tmp[:], in0=trans_s[:], scalar1=prev,
                                scalar2=None, op0=OP.add)
        tpsv = ps_small("tpsv")
        nc.tensor.transpose(tpsv[0:48, 0:48], tmp[:], I48)
        nc.vector.tensor_scalar(out=tmp2[:], in0=tpsv[0:48, 0:48],
                                scalar1=feats[:, t:t + 1],
                                scalar2=None, op0=OP.add)
        nc.vector.max(mx[:], tmp2[:])
        nc.vector.max_index(bps[:, t - 1:t + 7], mx[:], tmp2[:])
        prev = mx[:, 0:1]

    Vmat = scr.tile([48, S], fp32, name="Vmat", tag="sc8b")
    bps_f = scr.tile([48, S - 1], fp32, name="bps_f", tag="sc8c")
    lps = ps_small("lpsum")
    nc.tensor.matmul(lps[0:1, 0:48], mx[:, 0:1], I48, start=True, stop=True)
    nc.vector.tensor_copy(lastrow[:], lps[0:1, 0:48])
    nc.vector.max(lmax[:], lastrow[:])
    nc.vector.max_index(lidx[:], lmax[:], lastrow[:])
    nc.vector.tensor_copy(lidxf[:], lidx[:, 0:1])
    nc.vector.tensor_scalar(out=vrow[:], in0=iota_row, scalar1=lidxf[0:1, 0:1],
                            scalar2=None, op0=OP.is_equal)
    vcol = ps_small("vcol")
    nc.tensor.matmul(vcol[0:48, 0:1], vrow[:], one_1, start=True, stop=True)
    nc.vector.tensor_copy(Vmat[:, S - 1:S], vcol[0:48, 0:1])

    nc.vector.tensor_copy(bps_f[:], bps[:, 0:S - 1])

    tc.strict_bb_all_engine_barrier()

    # ---------------- Stage E ----------------
    for u in range(S - 1):
        t = S - 2 - u
        ms = Mscr[:, (u % 4) * 48:(u % 4) * 48 + 48]
        nc.vector.tensor_scalar(out=ms, in0=iotaM, scalar1=bps_f[:, t:t + 1],
                                scalar2=None, op0=OP.is_equal)
        vp = ps_small("vp")
        nc.tensor.matmul(vp[0:48, 0:1], ms, Vmat[:, t + 1:t + 2],
                         start=True, stop=True)
        nc.vector.tensor_copy(Vmat[:, t:t + 1], vp[0:48, 0:1])

    pathi = scr.tile([1, S], mybir.dt.int32, name="pathi", tag="sc8d")
    for hh_ in range(NH):
        pps = ps_big("ppsum")
        for nb in range(HWID // NBW):
            tb = hh_ * HWID + nb * NBW
            nc.tensor.matmul(pps[0:1, nb * NBW:(nb + 1) * NBW], iota_col,
                             Vmat[:, tb:tb + NBW], start=True, stop=True)
        nc.vector.tensor_copy(pathi[:, hh_ * HWID:(hh_ + 1) * HWID],
                              pps[0:1, 0:HWID])
    nc.sync.dma_start(io["out_path"], pathi[:])

    scr.release()
    psum.release()
    persist.release()


def emit_d2(nc, tc, io, S):
    """Viterbi forward only (core 0): per step = tensor_scalar add (t1 =
    trans + prev_col) -> PE transpose -> tensor_tensor_reduce (t2 = t1T +
    obs, alpha = rowmax) with max_index in the transpose shadow.
    Bit-exact with reference fp32 op order."""
    import concourse.mybir as mybir
    fp32 = mybir.dt.float32
    u16 = mybir.dt.uint16
    OP = mybir.AluOpType

    persist = tc.alloc_tile_pool(name="persist", bufs=1)
    pt = lambda shape, dt, name: persist.tile(shape, dt, name=name)
    consts = pt([128, 512], fp32, "consts")
    trans_s = pt([48, 48], fp32, "trans_s")
    feats = pt([48, S], fp32, "feats")
    alpha = pt([48, S], fp32, "alpha")
    bps = pt([48, 8 * (S - 1)], u16, "bps")
    I48 = consts[0:48, 2:50]

    scr = tc.alloc_tile_pool(name="scratch", bufs=1)
    psum = tc.alloc_tile_pool(name="psum", bufs=1, space="PSUM")

    nc.sync.dma_start(consts[:], io["consts"])
    nc.sync.dma_start(trans_s[:], io["trans"])
    nc.sync.dma_start(feats[:], io["feats_in"])
    nc.vector.memset(alpha[:], 0.0)

    tc.strict_bb_all_engine_barrier()

    nc.vector.tensor_copy(alpha[:, 0:1], feats[:, 0:1])
    hist = []
    for t in range(1, S):
        t1 = scr.tile([48, 48], fp32, name="t1", tag="t1", bufs=2)
        t2 = scr.tile([48, 48], fp32, name="t2", tag="t2", bufs=3)
        mx = scr.tile([48, 8], fp32, name="mx", tag="mx", bufs=3)
        tps = psum.tile([48, 48], fp32, name="tps", tag="tps", bufs=2)
        prev = alpha[:, 0:1] if t == 1 else hist[-1][0][:, 0:1]
        nc.vector.tensor_scalar(out=t1[:], in0=trans_s[:],
                                scalar1=prev, scalar2=None, op0=OP.add)
        if t > 1:
            # previous step's backpointer extraction hides in this step's
            # transpose window (DVE FIFO: after ts1, before next max8)
            pmx, pt2 = hist[-1]
            nc.vector.max_index(bps[:, 8 * (t - 2):8 * (t - 1)],
                                pmx[:], pt2[:])
        nc.tensor.transpose(tps[:], t1[:], I48)
        nc.vector.tensor_scalar(out=t2[:], in0=tps[:],
                                scalar1=feats[:, t:t + 1], scalar2=None,
                                op0=OP.add)
        nc.vector.max(mx[:], t2[:])
        if t == S - 1:
            nc.vector.tensor_copy(alpha[:, t:t + 1], mx[:, 0:1])
        hist.append((mx, t2))
    pmx, pt2 = hist[-1]
    nc.vector.max_index(bps[:, 8 * (S - 2):8 * (S - 1)], pmx[:], pt2[:])

    nc.sync.dma_start(io["alpha_out"], alpha[:])
    nc.sync.dma_start(io["bps_out"], bps[:])
    scr.release()
    psum.release()
    persist.release()


def emit_e2(nc, tc, io, L):
    """Backtrack chunk compose (SPMD x8): each core composes its chunk's
    48-entry backpointer maps (one-hot matmul chain) and logs the full
    trajectory P[u, e] = path state at local position u given entry e."""
    import concourse.mybir as mybir
    fp32 = mybir.dt.float32
    u16 = mybir.dt.uint16
    OP = mybir.AluOpType

    persist = tc.alloc_tile_pool(name="persist", bufs=1)
    pt = lambda shape, dt, name: persist.tile(shape, dt, name=name)
    consts = pt([128, 512], fp32, "consts")
    bpsu = pt([48, L], u16, "bpsu")
    bpf = pt([48, L], fp32, "bpf")
    msall = pt([48, 48 * L], fp32, "msall")
    P = pt([1, 48 * L], fp32, "P")
    Vs = [pt([48, 48], fp32, "V0"), pt([48, 48], fp32, "V1")]
    I48 = consts[0:48, 2:50]
    iotaM = consts[0:48, 240:288]
    iota_col = consts[0:48, 226:227]

    psum = tc.alloc_tile_pool(name="psum", bufs=1, space="PSUM")

    nc.sync.dma_start(consts[:], io["consts"])
    nc.sync.dma_start(bpsu[:], io["bps_in"])
    tc.strict_bb_all_engine_barrier()

    nc.vector.tensor_copy(bpf[:], bpsu[:])
    # precompute all one-hot ms matrices (no serial deps)
    for u in range(L):
        nc.vector.tensor_scalar(out=msall[:, 48 * u:48 * u + 48], in0=iotaM,
                                scalar1=bpf[:, u:u + 1], scalar2=None,
                                op0=OP.is_equal)
    # serial compose from the top; V_{L} = I48
    for k in range(L):
        u = L - 1 - k
        ms = msall[:, 48 * u:48 * u + 48]
        rhs = I48 if k == 0 else Vs[(k - 1) % 2][:]
        vps = psum.tile([48, 48], fp32, name="vps", tag="vps", bufs=2)
        sps = psum.tile([1, 48], fp32, name="sps", tag="sps", bufs=2)
        nc.tensor.matmul(vps[:], ms, rhs, start=True, stop=True)
        nc.vector.tensor_copy(Vs[k % 2][:], vps[:])
        nc.tensor.matmul(sps[:], iota_col, Vs[k % 2][:], start=True, stop=True)
        nc.vector.tensor_copy(P[:, 48 * u:48 * u + 48], sps[:])

    nc.sync.dma_start(io["p_out"], P[:])
    psum.release()
    persist.release()


def declare_io(nc, S, stages="ABC"):
    import concourse.mybir as mybir
    io = {}
    if "A" in stages:
        specs = [
            ("sent", (128, S // 128), mybir.dt.int32, "ExternalInput"),
            ("embed", (V, E), mybir.dt.float32, "ExternalInput"),
            ("WT_hi", (128, 2, 2048), mybir.dt.bfloat16, "ExternalInput"),
            ("WT_lo", (128, 2, 2048), mybir.dt.bfloat16, "ExternalInput"),
            ("WIT_hi", (128, 2, 2048), mybir.dt.bfloat16, "ExternalInput"),
            ("WIT_lo", (128, 2, 2048), mybir.dt.bfloat16, "ExternalInput"),
            ("bias_row", (1, 2048), mybir.dt.float32, "ExternalInput"),
            ("WOT_hi", (128, 4, 48), mybir.dt.bfloat16, "ExternalInput"),
            ("WOT_lo", (128, 4, 48), mybir.dt.bfloat16, "ExternalInput"),
            ("trans", (48, 48), mybir.dt.float32, "ExternalInput"),
            ("h0_init", (128, 8), mybir.dt.bfloat16, "ExternalInput"),
            ("c0_init", (4, 128), mybir.dt.float32, "ExternalInput"),
            ("consts", (128, 512), mybir.dt.float32, "ExternalInput"),
            ("zx", (S, 2048), mybir.dt.float32, "Internal"),
            ("feats_out", (48, S), mybir.dt.float32, "ExternalOutput"),
        ]
    elif stages == "D2":
        specs = [
            ("consts", (128, 512), mybir.dt.float32, "ExternalInput"),
            ("trans", (48, 48), mybir.dt.float32, "ExternalInput"),
            ("feats_in", (48, S), mybir.dt.float32, "ExternalInput"),
            ("alpha_out", (48, S), mybir.dt.float32, "ExternalOutput"),
            ("bps_out", (48, 8 * (S - 1)), mybir.dt.uint16, "ExternalOutput"),
        ]
    elif stages == "E2":
        L = S  # reuse S slot as chunk length
        specs = [
            ("consts", (128, 512), mybir.dt.float32, "ExternalInput"),
            ("bps_in", (48, L), mybir.dt.uint16, "ExternalInput"),
            ("p_out", (1, 48 * L), mybir.dt.float32, "ExternalOutput"),
        ]
    else:
        specs = [
            ("consts", (128, 512), mybir.dt.float32, "ExternalInput"),
            ("trans", (48, 48), mybir.dt.float32, "ExternalInput"),
            ("feats_in", (48, S), mybir.dt.float32, "ExternalInput"),
            ("out_path", (1, S), mybir.dt.int32, "ExternalOutput"),
        ]
    for nm, shp, dt, kind in specs:
        io[nm] = nc.dram_tensor(nm, shp, dt, kind=kind).ap()
    return io


def build_nc(S, WIN, stages="ABC", debug=False):
    import concourse.bacc as bacc
    from concourse import tile
    nc = bacc.Bacc("TRN2", target_bir_lowering=False, debug=debug)
    io = declare_io(nc, S, stages)
    with tile.TileContext(nc) as tc:
        if "A" in stages:
            emit(nc, tc, io, S, WIN)
        elif stages == "D2":
            emit_d2(nc, tc, io, S)
        elif stages == "E2":
            emit_e2(nc, tc, io, S)
        else:
            emit_de(nc, tc, io, S)
    nc.compile()
    return nc


LAST_EXEC_NS = None
LAST_EXEC_PARTS = None


def _ensure_ntff_hook():
    """This image's antenv lacks axon_hooks; synthesize it and install the
    ctypes-based NTFF profile hook from trn_agent_boot so trace=True works."""
    try:
        from antenv.axon_hooks import get_axon_ntff_profile_hook
        if get_axon_ntff_profile_hook() is not None:
            return True
    except ImportError:
        import sys as _s, types as _t
        try:
            import antenv
        except ImportError:
            return False
        mod = _t.ModuleType("antenv.axon_hooks")
        mod._hook = None
        mod.set_axon_ntff_profile_hook = lambda h: setattr(mod, "_hook", h)
        mod.get_axon_ntff_profile_hook = lambda: mod._hook
        _s.modules["antenv.axon_hooks"] = mod
        antenv.axon_hooks = mod
    try:
        from antenv.axon_hooks import (get_axon_ntff_profile_hook,
                                       set_axon_ntff_profile_hook)
        if get_axon_ntff_profile_hook() is None:
            from trn_agent_boot.trn_boot import _ntff_profile_via_ctypes
            set_axon_ntff_profile_hook(
                _ntff_profile_via_ctypes('/opt/axon/libaxon_pjrt.so'))
        return get_axon_ntff_profile_hook() is not None
    except Exception:
        return False


_SPMD_CALLS = 0


def _run_spmd(nc, in_maps, core_ids):
    """run_bass_kernel_spmd with trace (for exec_time_ns); falls back to
    untraced on any profiling failure."""
    import os
    global _SPMD_CALLS
    _SPMD_CALLS += 1
    from concourse import bass_utils
    tmp = os.environ.get("KTRACE_DIR")
    if tmp:
        tmp = os.path.join(tmp, f"l{_SPMD_CALLS}")
        os.makedirs(tmp, exist_ok=True)
    try:
        if not _ensure_ntff_hook():
            raise RuntimeError("no ntff hook")
        return bass_utils.run_bass_kernel_spmd(
            nc, in_maps, core_ids=core_ids, trace=True,
            tmpdir=(tmp if tmp else None))
    except Exception as e:
        print(f"traced run failed ({type(e).__name__}: {e}); retrying untraced")
        return bass_utils.run_bass_kernel_spmd(nc, in_maps, core_ids=core_ids)


def kernel(**inputs):
    """Chunked-SPMD BiLSTM-CRF in two launches.

    Launch 1: each of the 8 cores runs the BiLSTM + emission projection for
    one 256-step slice using a 384-step window (the LSTM forget gate makes a
    44-100 step warmup converge to the full-sequence hidden state to ~2e-7);
    the host stitches the per-core feats into the full (48, 2048) matrix.

    Launch 2: exact full-length Viterbi decode + backtrack on the stitched
    feats (single core) - reproduces the reference's fp32 tie-breaking, which
    windowed alphas cannot."""
    global LAST_EXEC_NS
    from concourse import bass_utils
    Sfull = int(np.asarray(inputs["sent"]).shape[0])
    NC = 8
    L = Sfull // NC          # 256 output steps per core
    SW_ = 384                # window length per core
    shared = prep_shared(inputs)
    SHIFT = 84               # fwd warmup 84, bwd warmup 44 steps
    in_maps, offs = [], []
    for c in range(NC):
        ws = min(max(c * L - SHIFT, 0), Sfull - SW_)
        dev = dict(shared)
        dev.update(prep_core(inputs, ws, SW_,
                             fwd_real=(ws == 0),
                             bwd_real=(ws + SW_ == Sfull)))
        in_maps.append(dev)
        offs.append(c * L - ws)
    nc1 = build_nc(SW_, WIN=4, stages="ABC")
    res1 = _run_spmd(nc1, in_maps, core_ids=list(range(NC)))
    feats = np.zeros((48, Sfull), np.float32)
    for c in range(NC):
        fw = np.asarray(res1.results[c]["feats_out"], np.float32)
        feats[:, c * L:(c + 1) * L] = fw[:, offs[c]:offs[c] + L]

    global LAST_EXEC_PARTS
    try:
        # Launch 2: Viterbi forward on core 0 (alphas + raw backpointers).
        nc2 = build_nc(Sfull, WIN=4, stages="D2")
        dev2 = {"consts": shared["consts"], "trans": shared["trans"],
                "feats_in": np.ascontiguousarray(feats)}
        res2 = _run_spmd(nc2, [dev2], core_ids=[0])
        alpha = np.asarray(res2.results[0]["alpha_out"], np.float32)
        bps8 = np.asarray(res2.results[0]["bps_out"])    # (48, 8*(S-1)) u16
        bp = bps8.reshape(48, Sfull - 1, 8)[:, :, 0]     # ref_bps[i] = bp[:, i]

        # Launch 3: 8-core parallel backtrack compose (48-entry trajectories).
        L = 256
        starts = [c * L for c in range(7)] + [Sfull - 1 - L]
        nc3 = build_nc(L, WIN=4, stages="E2")
        in3 = [{"consts": shared["consts"],
                "bps_in": np.ascontiguousarray(bp[:, s:s + L].astype(np.uint16))}
               for s in starts]
        res3 = _run_spmd(nc3, in3, core_ids=list(range(8)))

        last = int(np.argmax(alpha[:, -1]))
        path = np.zeros(Sfull, np.int64)
        path[Sfull - 1] = last
        for c in range(7, -1, -1):
            s0 = starts[c]
            e = last if c == 7 else int(path[s0 + L])
            P = np.asarray(res3.results[c]["p_out"], np.float32).reshape(L, 48)
            path[s0:s0 + L] = P[:, e].astype(np.int64)
        t1 = res1.exec_time_ns
        t2 = res2.exec_time_ns
        t3 = res3.exec_time_ns
        LAST_EXEC_PARTS = (t1, t2, t3)
        parts = [t1, t2, t3]
        LAST_EXEC_NS = sum(parts) if all(p is not None for p in parts) else None
        return path.astype(np.int32)
    except Exception as e:
        print(f"fast viterbi path failed ({type(e).__name__}: {e}); "
              f"falling back to single-core DE")
        nc2 = build_nc(Sfull, WIN=4, stages="DE")
        dev2 = {"consts": shared["consts"], "trans": shared["trans"],
                "feats_in": np.ascontiguousarray(feats)}
        res2 = _run_spmd(nc2, [dev2], core_ids=[0])
        t1 = res1.exec_time_ns
        t2 = res2.exec_time_ns
        LAST_EXEC_PARTS = (t1, t2)
        LAST_EXEC_NS = (t1 + t2) if (t1 is not None and t2 is not None) else None
        return res2.results[0]["out_path"].reshape(Sfull).astype(np.int32)

